# revision 2
# baseline (speedup 1.0000x reference)
"""Trainium2 Bass kernel for nn_CrossAttention (B=4, S=1024, C=1024, H=16).

Sharding: 8 cores = (batch b in 0..4) x (query-half qs in 0..2). Each core
computes, for its 512 query rows of batch b: the Q projection, K/V
projections over the batch's *valid* key positions, masked-softmax
attention over all 16 heads, the output projection, and the MLP with
residual. No collectives.

Key structure (v4 of this kernel):
- The key mask is identical for every query row of a batch, so the host
  gathers only the valid key columns (mask OR over the two modalities) and
  pads to SV (a multiple of 128; 896 for the reference mask). All
  K/QK/V/PV work shrinks from S=1024 to SV.
- Pad K columns are host-written as exact duplicates of valid column 0,
  so their logits equal column 0's logits (no overflow, no masking work);
  pad V columns are zero so they don't touch the numerator. The softmax
  denominator is corrected by subtracting npad * p0, where p0 is
  partition 0 of the head's already-computed exp tile.
- Q/K biases are folded into the eviction tensor_scalar ops.
- QK matmuls use a 65-row contraction (64 head dims + one zeroed dummy
  row) so the PE stays in the full 128x128 tiling mode.
- The Q/K projections and both MLP matmuls run in fp8(e4m3) DoubleRow
  mode: the host ships q/k inputs and Wq/Wk/W1/W2 pre-scaled by powers
  of two (exact to undo); evictions fold the descale into their bias
  ops; gelu un-scales via the ACT scale; the final eviction un-scales
  the W2 product while adding the residual. V/P stay bf16 (fp8 there
  pushes rel_err too close to the 2e-2 gate).
- Host pre-swizzles all bulk tensors into exact SBUF slab layouts so DMA
  moves whole-tensor multi-KB segments per partition. Loads are issued
  in global need order across the three DMA-capable queues (sync,
  scalar, gpsimd) so the rings run at full rate from boot: wq+qt first
  (Q proj), then kin+wk (K proj), then vt+wv, then wp/w1/w2.
- The vaug ones-memset runs on gpsimd (it used to occupy the DVE for
  6us at boot and stall the Q-proj evictions).
- All activations flow transposed (contraction dim on SBUF partitions):
    qTa[0:64, h, :]  = (Wq'^T qT) / SQ2 + bq'     (' = *SCALE)
    kTa[0:64, h, :]  = (Wk^T kT + bk*SK2) / SK2
    LT_h = kTa_h^T @ qTa_h                [kpos, q]
    PT_h = exp(LT_h)
    oT_h = [v_h | 1]^T @ PT_h             [65, q]; row 64 = denom_raw
    xT[h] = oT_h[0:64] * bcast(1/(denom_raw - npad*p0))
    xpT   = Wp^T xT + bp_eff (+b2 in the residual copy)
    h1T   = gelu((W1s^T xp8) * inv1 + b1)
    outT  = xpT_res + (W2s^T h18) * inv2
"""

from contextlib import ExitStack

import numpy as np

import concourse.bass as bass
import concourse.tile as tile
from concourse import bacc, mybir
from concourse.bass_utils import run_bass_kernel_spmd

B, S, C, H = 4, 1024, 1024, 16
HD = C // H          # 64
SCALE = HD ** -0.5
P = 128              # SBUF partitions
SQ = S // 2          # 512 query rows per core
NCORES = 8
KT = C // P          # 8 contraction tiles of 128
N512 = 512

SXP = 1024.0         # fp8 scale for xp (M1 input)
SH = 2048.0          # fp8 scale for h1 (M2 input)
SW = 4096.0          # fp8 scale for W1/W2
INV1 = 1.0 / (SXP * SW)
INV2 = 1.0 / (SH * SW)
SQI = 32.0           # fp8 scale for the q/k inputs
SWQ = 8192.0         # fp8 scale for Wq*SCALE
SWK = 4096.0         # fp8 scale for Wk
SQ2 = SQI * SWQ      # q-proj psum scale (2^18)
SK2 = SQI * SWK      # k-proj psum scale (2^17)

F32 = mybir.dt.float32
BF16 = mybir.dt.bfloat16
FP8 = mybir.dt.float8e4
NPBF16 = mybir.dt.np(BF16)
NPFP8 = mybir.dt.np(FP8)
FP8MAX = 240.0


def build_program(SV):
    KTS = SV // P        # key-position tiles (7 for SV=896)
    NK = SV // 2         # K-proj eviction chunk width

    nc = bacc.Bacc(None, target_bir_lowering=False, debug=False)

    # all bulk inputs arrive pre-swizzled by the host into their exact SBUF
    # slab layout [128 partitions, free bytes] so every DMA moves one long
    # contiguous segment per partition
    wq = nc.dram_tensor("wq", [P, KT * C], FP8, kind="ExternalInput")
    wk = nc.dram_tensor("wk", [P, KT * C], FP8, kind="ExternalInput")
    wv = nc.dram_tensor("wv", [P, KT * C], BF16, kind="ExternalInput")
    wp = nc.dram_tensor("wp", [P, KT * C], BF16, kind="ExternalInput")
    w1 = nc.dram_tensor("w1", [P, KT * C], FP8, kind="ExternalInput")
    w2 = nc.dram_tensor("w2", [P, KT * C], FP8, kind="ExternalInput")
    qt_in = nc.dram_tensor("qt_in", [P, KT * SQ], FP8, kind="ExternalInput")
    kt_in = nc.dram_tensor("kt_in", [P, KT * SV], FP8, kind="ExternalInput")
    vt_in = nc.dram_tensor("vt_in", [P, KT * SV], BF16, kind="ExternalInput")
    # a zeros row DMA'd into partition 64 of qTa/kTa (their dummy
    # contraction row); a strided memset there costs microseconds on DVE
    zrow = nc.dram_tensor("zrow", [1, H * SV], BF16, kind="ExternalInput")
    # per-channel vectors packed to [P, 6, KT]:
    # i=0..4 -> bq'*SQ2, bk*SK2, b1, bp_eff+b2, b2; [0, 5, 0] = -npad
    bvecs = nc.dram_tensor("bvecs", [P, 6, KT], F32, kind="ExternalInput")
    out = nc.dram_tensor("out", [C, SQ], F32, kind="ExternalOutput")

    add = mybir.AluOpType.add
    mult = mybir.AluOpType.mult
    subtract = mybir.AluOpType.subtract
    Act = mybir.ActivationFunctionType
    DR = mybir.MatmulPerfMode.DoubleRow

    with tile.TileContext(nc) as tc, ExitStack() as ctx:
        const = ctx.enter_context(tc.tile_pool(name="const", bufs=1))
        wfull = ctx.enter_context(tc.tile_pool(name="wfull", bufs=2))
        acts = ctx.enter_context(tc.tile_pool(name="acts", bufs=1))
        ptp = ctx.enter_context(tc.tile_pool(name="ptp", bufs=5))
        smal = ctx.enter_context(tc.tile_pool(name="smal", bufs=2))
        outp = ctx.enter_context(tc.tile_pool(name="outp", bufs=3))
        ps = ctx.enter_context(tc.tile_pool(name="ps", bufs=2, space="PSUM"))
        pslt = ctx.enter_context(tc.tile_pool(name="pslt", bufs=3, space="PSUM"))

        # ---- constants ----
        bv_sb = const.tile([P, 6, KT], F32, tag="bvecs")
        bq_sb = bv_sb[:, 0, :]
        bk_sb = bv_sb[:, 1, :]
        b1_sb = bv_sb[:, 2, :]
        bpb2_sb = bv_sb[:, 3, :]
        b2_sb = bv_sb[:, 4, :]
        nnpad_sb = bv_sb[0:1, 5, 0:1]          # -npad

        # ---- input activations (resident) ----
        qin = acts.tile([P, KT, SQ], FP8, tag="qin_xT")
        kin = acts.tile([P, KT, SV], FP8, tag="kin_h1T")
        vin = acts.tile([P, KT, SV], BF16, tag="vin_xpT")

        # ---- intermediates, resident ----
        # qTa/kTa: rows 0-63 head data, row 64 zeroed (keeps the QK
        # contraction at 65 rows -> full 128x128 PE tiling mode)
        qTa = acts.tile([P, H, SQ], BF16, tag="qTa")
        kTa = acts.tile([P, H, SV], BF16, tag="kTa")
        vaug = acts.tile([P, KTS, H * 65], BF16, tag="va")  # [kpos, h*(64|1)]
        xT = acts.tile([P, KT, SQ], BF16, tag="qin_xT")        # [c, q] attn out
        xpT = acts.tile([P, KT, SQ], BF16, tag="vin_xpT")      # [c', q] resid
        h1T = acts.tile([P, KT, SQ], BF16, tag="kin_h1T")      # [c_h, q] hidden
        xp8 = acts.tile([P, KT, SQ], FP8, tag="xp8")           # scaled M1 input
        h18 = acts.tile([P, KT, SQ], FP8, tag="h18")           # scaled M2 input

        vaug_h = vaug.rearrange("p k (h e) -> p k h e", e=65)

        def rearr(src, n):
            return src.rearrange("p (k n) -> p k n", n=n)

        wq_r, wk_r, wv_r = rearr(wq, C), rearr(wk, C), rearr(wv, C)
        wp_r, w1_r, w2_r = rearr(wp, C), rearr(w1, C), rearr(w2, C)
        qt_r, kt_r, vt_r = rearr(qt_in, SQ), rearr(kt_in, SV), rearr(vt_in, SV)

        # ---- weight slabs (2-slot rotation: A=wq->wv->w1, B=wk->wp->w2) ----
        wsb_q = wfull.tile([P, KT, C], FP8, tag="w8")
        wsb_k = wfull.tile([P, KT, C], FP8, tag="w8")

        # ---- DMA issue, global need order across the 3 queues ----
        # sync:   wq[0:2], wq[2:8], kin, vt, (wp, w1 after slots free), out
        # scalar: qt[0:2], qt[2:8], bvecs, then all ACT work
        # gpsimd: wk, zrows, memset(vaug), wv (waits Q-proj end), w2
        nc.sync.dma_start(wsb_q[:, 0:2, :], wq_r[:, 0:2, :])
        nc.scalar.dma_start(qin[:, 0:2, :], qt_r[:, 0:2, :])
        nc.gpsimd.dma_start(wsb_k[:, :, :], wk_r[:, :, :])
        nc.sync.dma_start(wsb_q[:, 2:KT, :], wq_r[:, 2:KT, :])
        nc.scalar.dma_start(qin[:, 2:KT, :], qt_r[:, 2:KT, :])
        nc.scalar.dma_start(bv_sb[:, :, :], bvecs[:, :, :])
        nc.sync.dma_start(kin[:, :, :], kt_r[:, :, :])
        nc.gpsimd.dma_start(
            qTa[HD:HD + 1, :, :],
            zrow[0:1, 0:H * SQ].rearrange("o (h n) -> o h n", n=SQ))
        nc.gpsimd.dma_start(
            kTa[HD:HD + 1, :, :],
            zrow[0:1, :].rearrange("o (h n) -> o h n", n=SV))
        nc.sync.dma_start(vin[:, :, :], vt_r[:, :, :])
        # one contiguous memset covers the per-head ones-columns; the V
        # evictions overwrite every data column afterwards
        nc.gpsimd.memset(vaug[:, :, :], 1.0)

        # ---- QK work queue: units of (head, ktile-pair). Each unit is 1-2
        # matmuls into one [P, 2*N512] psum tile plus one exp ACT covering
        # both ktiles. Drained a few units at a time between other PE work
        # so the ScalarEngine's exp stream paces evenly. ----
        pTts = {}
        qk_tasks = []
        NUNIT = (KTS + 1) // 2

        def enqueue_qk(h):
            pTt = ptp.tile([P, KTS, N512], BF16, tag="pt")
            pTts[h] = pTt
            for u in range(NUNIT):
                qk_tasks.append((h, u))

        def drain_qk(n):
            for _ in range(min(n, len(qk_tasks))):
                h, u = qk_tasks.pop(0)
                kts = list(range(2 * u, min(2 * u + 2, KTS)))
                lt = pslt.tile([P, 2 * N512], F32, tag="lt")
                for j, kt in enumerate(kts):
                    nc.tensor.matmul(
                        lt[:, j * N512:(j + 1) * N512],
                        kTa[0:HD + 1, h, kt * P:(kt + 1) * P],
                        qTa[0:HD + 1, h, :],
                        start=True, stop=True,
                    )
                nj = len(kts)
                nc.scalar.activation(
                    out=pTts[h][:, 2 * u:2 * u + nj, :],
                    in_=lt[:, 0:nj * N512].rearrange("p (t n) -> p t n", n=N512),
                    func=Act.Exp,
                )

        # ---- Q projection (fp8 DoubleRow; descale+bias in the eviction) ----
        wsb = wsb_q
        for m in range(KT):
            pt = ps.tile([P, N512], F32, tag="mm")
            for d in range(KT // 2):
                nc.tensor.matmul(
                    pt[:, :],
                    wsb[:, 2 * d:2 * d + 2, m * P:(m + 1) * P],
                    qin[:, 2 * d:2 * d + 2, :],
                    start=(d == 0), stop=(d == KT // 2 - 1),
                    perf_mode=DR,
                )
            for j in range(2):
                nc.vector.tensor_scalar(
                    out=qTa[0:HD, 2 * m + j, :], in0=pt[j * HD:(j + 1) * HD, :],
                    scalar1=bq_sb[j * HD:(j + 1) * HD, m:m + 1],
                    scalar2=1.0 / SQ2, op0=add, op1=mult,
                )

        # wv reuses slot A (waits on Q-proj last read of wq); w2 later
        wsb_v = wfull.tile([P, KT, C], BF16, tag="w8")
        nc.gpsimd.dma_start(wsb_v[:, :, :], wv_r[:, :, :])

        # ---- K projection (fp8 DoubleRow); QK starts as kTa lands ----
        # eviction: (psum + bk*SK2) / SK2; pad columns are host-duplicated
        # from valid column 0, so no masking is needed
        wsb = wsb_k
        for m in range(KT):
            for n in range(2):
                pt = ps.tile([P, NK], F32, tag="mm")
                for d in range(KT // 2):
                    nc.tensor.matmul(
                        pt[:, :],
                        wsb[:, 2 * d:2 * d + 2, m * P:(m + 1) * P],
                        kin[:, 2 * d:2 * d + 2, n * NK:(n + 1) * NK],
                        start=(d == 0), stop=(d == KT // 2 - 1),
                        perf_mode=DR,
                    )
                ns = slice(n * NK, (n + 1) * NK)
                for j in range(2):
                    nc.vector.tensor_scalar(
                        out=kTa[0:HD, 2 * m + j, ns],
                        in0=pt[j * HD:(j + 1) * HD, :],
                        scalar1=bk_sb[j * HD:(j + 1) * HD, m:m + 1],
                        scalar2=1.0 / SK2, op0=add, op1=mult,
                    )
            enqueue_qk(2 * m)
            enqueue_qk(2 * m + 1)
            drain_qk(2)

        # wp reuses slot B (waits on K-proj last read of wk)
        wsb_p = wfull.tile([P, KT, C], BF16, tag="w8")
        nc.sync.dma_start(wsb_p[:, :, :], wp_r[:, :, :])

        # ---- attention: V projection + remaining QK + PV pipeline ----
        wsb = wsb_v

        def emit_v_chunk(i):
            # kpos tile m, c_out chunk n
            m, n = i % KTS, i // KTS
            pt = ps.tile([P, N512], F32, tag="mm")
            for k in range(KT):
                nc.tensor.matmul(
                    pt[:, :],
                    vin[:, k, m * P:(m + 1) * P],
                    wsb[:, k, n * N512:(n + 1) * N512],
                    start=(k == 0), stop=(k == KT - 1),
                )
            nc.vector.tensor_copy(
                vaug_h[:, m, 8 * n:8 * n + 8, 0:64],
                pt[:, :].rearrange("p (h d) -> p h d", d=HD),
            )

        def emit_pv(h):
            hp = (h % 2) * HD
            hm = h // 2
            pTt = pTts.pop(h)
            pv = ps.tile([HD + 1, N512], F32, tag="mm")
            for kt in range(KTS):
                nc.tensor.matmul(
                    pv[:, :],
                    vaug[:, kt, h * 65:(h + 1) * 65],
                    pTt[:, kt, :],
                    start=(kt == 0), stop=(kt == KTS - 1),
                )
            rc = smal.tile([1, N512], F32, tag="rc")
            bc = smal.tile([HD, N512], F32, tag="bc")
            # denom_raw = true_denom + npad*p0 (pad K cols duplicate valid
            # col 0; pad V cols are zero). p0 is partition 0 of the exp'd
            # logits, ktile 0. true_denom >= p_max > 0, so the fast
            # reciprocal's denorm/zero edge cases cannot occur
            nc.vector.scalar_tensor_tensor(
                out=rc[0:1, :], in0=pTt[0:1, 0, :],
                scalar=nnpad_sb, in1=pv[HD:HD + 1, :],
                op0=mult, op1=add,
            )
            nc.vector.reciprocal_approx_fast(out=rc[0:1, :], in_=rc[0:1, :])
            nc.gpsimd.partition_broadcast(bc[:, :], rc[0:1, :])
            nc.vector.tensor_mul(xT[hp:hp + HD, hm, :], pv[0:HD, :], bc[:, :])

        # V chunks n=0 first (PV_0..7 read the full n=0 group of vaug),
        # then the PV pipeline with the remaining QKs and V chunks
        # interleaved.
        for i in range(KTS):
            emit_v_chunk(i)
            drain_qk(2)
        for h in range(H):
            emit_pv(h)
            drain_qk(2)
            if h < KTS:
                emit_v_chunk(KTS + h)
                drain_qk(1)
        drain_qk(len(qk_tasks))

        # w1 reuses slot A (waits on V-proj last read of wv)
        wsb_1 = wfull.tile([P, KT, C], FP8, tag="w8")
        nc.sync.dma_start(wsb_1[:, :, :], w1_r[:, :, :])

        # ---- output projection (bf16 residual + fp8 copy) ----
        wsb = wsb_p
        for m in range(KT):
            pt = ps.tile([P, N512], F32, tag="mm")
            for k in range(KT):
                nc.tensor.matmul(
                    pt[:, :], wsb[:, k, m * P:(m + 1) * P], xT[:, k, :],
                    start=(k == 0), stop=(k == KT - 1),
                )
            nc.vector.tensor_scalar(
                out=xpT[:, m, :], in0=pt[:, :],
                scalar1=bpb2_sb[:, m:m + 1], scalar2=None, op0=add,
            )
            nc.vector.tensor_scalar(
                out=xp8[:, m, :], in0=xpT[:, m, :],
                scalar1=b2_sb[:, m:m + 1], scalar2=SXP, op0=subtract,
                op1=mult,
            )

        # w2 reuses slot B (waits on P-proj last read of wp)
        wsb_2 = wfull.tile([P, KT, C], FP8, tag="w8")
        nc.gpsimd.dma_start(wsb_2[:, :, :], w2_r[:, :, :])

        # ---- MLP in fp8 DoubleRow ----
        wsb = wsb_1
        for m in range(KT):
            pt = ps.tile([P, N512], F32, tag="mm")
            for d in range(KT // 2):
                nc.tensor.matmul(
                    pt[:, :],
                    wsb[:, 2 * d:2 * d + 2, m * P:(m + 1) * P],
                    xp8[:, 2 * d:2 * d + 2, :],
                    start=(d == 0), stop=(d == KT // 2 - 1),
                    perf_mode=DR,
                )
            nc.scalar.activation(
                out=h1T[:, m, :], in_=pt[:, :], func=Act.Gelu,
                bias=b1_sb[:, m:m + 1], scale=INV1,
            )
            nc.vector.tensor_scalar(
                out=h18[:, m, :], in0=h1T[:, m, :],
                scalar1=SH, scalar2=None, op0=mult,
            )

        wsb = wsb_2
        for m in range(KT):
            pt = ps.tile([P, N512], F32, tag="mm")
            for d in range(KT // 2):
                nc.tensor.matmul(
                    pt[:, :],
                    wsb[:, 2 * d:2 * d + 2, m * P:(m + 1) * P],
                    h18[:, 2 * d:2 * d + 2, :],
                    start=(d == 0), stop=(d == KT // 2 - 1),
                    perf_mode=DR,
                )
            # split the final tile's eviction+store so the tail overlaps
            halves = (2,) if m == KT - 1 else (1,)
            nh = halves[0]
            for hh in range(nh):
                hs = slice(hh * N512 // nh, (hh + 1) * N512 // nh)
                ot = outp.tile([P, N512 // nh], F32, tag=f"o{nh}{hh}")
                nc.vector.scalar_tensor_tensor(
                    out=ot[:, :], in0=pt[:, hs], scalar=INV2,
                    in1=xpT[:, m, hs], op0=mult, op1=add,
                )
                nc.sync.dma_start(
                    out[m * P:(m + 1) * P, hs], ot[:, :])

    nc.compile()
    return nc


_prog_cache = {}


def _get_program(SV):
    if SV not in _prog_cache:
        _prog_cache[SV] = build_program(SV)
    return _prog_cache[SV]


def make_in_maps(inputs, SV, valid_idx, nvalid):
    q = np.asarray(inputs["query"], np.float32)
    k = np.asarray(inputs["key"], np.float32)
    v = np.asarray(inputs["value"], np.float32)
    Wq = np.asarray(inputs["Wq"], np.float32) * SCALE
    bq = np.asarray(inputs["bq"], np.float32) * SCALE
    Wk = np.asarray(inputs["Wk"], np.float32)
    bk = np.asarray(inputs["bk"], np.float32)
    Wv = np.asarray(inputs["Wv"], np.float32)
    bv = np.asarray(inputs["bv"], np.float32)
    Wp = np.asarray(inputs["Wp"], np.float32)
    bp = np.asarray(inputs["bp"], np.float32)
    W1 = np.asarray(inputs["W1"], np.float32)
    b1 = np.asarray(inputs["b1"], np.float32)
    W2 = np.asarray(inputs["W2"], np.float32)
    b2 = np.asarray(inputs["b2"], np.float32)

    bp_eff = bp + bv @ Wp

    def swizzle(w):  # [C=KT*P, N] -> SBUF slab [P, KT*N]
        n = w.shape[1]
        return np.ascontiguousarray(
            w.reshape(KT, P, n).transpose(1, 0, 2).reshape(P, KT * n))

    def to_fp8(w, s):
        return swizzle(np.clip(w * s, -FP8MAX, FP8MAX).astype(NPFP8))

    shared = {
        "wq": to_fp8(Wq, SWQ),
        "wk": to_fp8(Wk, SWK),
        "wv": swizzle(Wv.astype(NPBF16)),
        "wp": swizzle(Wp.astype(NPBF16)),
        "w1": to_fp8(W1, SW),
        "w2": to_fp8(W2, SW),
    }

    def pack_cols(vec):      # [C] -> [P, KT] with [p, j] = vec[j*128+p]
        return np.asarray(vec, np.float32).reshape(KT, P).T

    in_maps = []
    for core in range(NCORES):
        b, qs = divmod(core, 2)
        nv = int(nvalid[b])
        idx = valid_idx[b]
        m = dict(shared)
        m["qt_in"] = swizzle(np.clip(
            q[b, qs * SQ:(qs + 1) * SQ, :].T * SQI,
            -FP8MAX, FP8MAX).astype(NPFP8))
        ktg = np.empty((C, SV), NPFP8)
        ktg[:, :nv] = np.clip(
            k[b].T[:, idx] * SQI, -FP8MAX, FP8MAX).astype(NPFP8)
        # pad K columns duplicate valid column 0: their logits match
        # column 0's, and the denominator correction subtracts npad*p0
        ktg[:, nv:] = ktg[:, 0:1]
        vtg = np.zeros((C, SV), NPBF16)
        vtg[:, :nv] = v[b].T[:, idx].astype(NPBF16)
        m["kt_in"] = swizzle(ktg)
        m["vt_in"] = swizzle(vtg)
        m["zrow"] = np.zeros((1, H * SV), NPBF16)
        base = np.zeros((P, 6, KT), np.float32)
        for i, vec in enumerate(
                (bq * SQ2, bk * SK2, b1, bp_eff + b2, b2)):
            base[:, i, :] = pack_cols(vec)
        base[0, 5, 0] = -float(SV - nv)
        m["bvecs"] = np.ascontiguousarray(base)
        in_maps.append(m)
    return in_maps


def run(inputs, trace=False, trace_cores=None):
    mask = np.asarray(inputs["mask"])
    combined = (mask[:, :S] != 0) | (mask[:, S:2 * S] != 0)   # [B, S]
    valid_idx = [np.nonzero(combined[b])[0] for b in range(B)]
    nvalid = np.array([len(ix) for ix in valid_idx])
    SV = max(P, int(-(-int(nvalid.max()) // P)) * P)
    nc = _get_program(SV)
    in_maps = make_in_maps(inputs, SV, valid_idx, nvalid)
    res = run_bass_kernel_spmd(
        nc, in_maps, core_ids=list(range(NCORES)),
        trace=trace, trace_cores=trace_cores,
    )
    outfull = np.empty((B, S, C), np.float32)
    for core in range(NCORES):
        b, qs = divmod(core, 2)
        outfull[b, qs * SQ:(qs + 1) * SQ, :] = res.results[core]["out"].T
    return outfull, res


def kernel(**inputs):
    outfull, _ = run(inputs)
    return outfull
